# revision 46
# baseline (speedup 1.0000x reference)
"""Distributed Trainium2 kernel for nn_Attention_81028853007052.

8 cores = batch(2) x 4 query-block groups. Core (b, qc) processes the four
interleaved 128-row query blocks {qc, 4+qc, 8+qc, 12+qc} of batch b; slot s
(local block s, global block 4s+qc) attends keys [0, 512(s+1)+2) -- causally
balanced and SPMD-uniform. Per-row causal thresholds are passed as data.

Internal key layout: col 0,1 = null kv; cols 2..127 dead padding; col 128+j =
x-key j (ref col j+2). thresh' = ref_thresh + 126 compares directly against
internal col index.

Everything is computed in transposed ("T") layouts so no PE transposes are
needed and every matmul has a wide (>=256) moving dim:
  - qT = Wq^T @ (x - mu)^T directly from host-transposed x; the layernorm
    rstd cancels inside the per-head l2norm so only the mean is subtracted.
  - l2norm partition-dim sums of squares via ones-vector matmuls.
  - scores^T[key, row] accumulated per 128-key chunk; even/odd head pairs
    issued back-to-back at partition bases 0/64 for row-tile concurrency.
  - exp on ScalarE with a per-partition bias that kills the dead padding
    keys of chunk 0; causal diag masks (0/1, data-driven thresholds) are
    multiplied on GpSimd which is otherwise idle.
  - attn@V flipped: out^T[c, row] = V^T @ es accumulated over chunks, with
    a ones-column in V giving the softmax denominator.
  - out^T feeds the output projection directly (it is already the lhsT).
All matmuls run bf16 (1 cycle/row); accumulation is fp32 in PSUM.
"""

import numpy as np
from contextlib import ExitStack

import concourse.bass as bass
import concourse.mybir as mybir
import concourse.tile as tile
from concourse import bacc
from concourse.bass_utils import run_bass_kernel_spmd
from concourse.masks import make_identity

P = 128
D = 1024
H = 16
DH = 64
R = 512          # query rows per core
NB = 4           # local query blocks (= slots)
NCH = 17         # key chunks of 128 (1 null/pad chunk + 16 x chunks)
NKEY = NCH * P   # 2176
F32 = mybir.dt.float32
BF16 = mybir.dt.bfloat16
AF = mybir.ActivationFunctionType
AL = mybir.AluOpType
X = mybir.AxisListType.X

_CACHE = {}


def _install_ntff_hook():
    """Best-effort: register the axon NTFF profile hook so trace=True works."""
    import sys
    if "antenv.axon_hooks" in sys.modules:
        return True
    try:
        import contextlib
        import ctypes
        import types

        lib = ctypes.CDLL("/opt/axon/libaxon_pjrt.so")
        if not hasattr(lib, "axon_start_nrt_profile"):
            return False
        lib.axon_start_nrt_profile.argtypes = [
            ctypes.POINTER(ctypes.c_int64), ctypes.c_size_t]
        lib.axon_start_nrt_profile.restype = ctypes.c_int64
        lib.axon_stop_nrt_profile.argtypes = [ctypes.c_char_p]
        lib.axon_stop_nrt_profile.restype = ctypes.c_int64

        @contextlib.contextmanager
        def _hook(output_dir, device_ids):
            import jax
            jax.devices()
            if device_ids:
                ids = (ctypes.c_int64 * len(device_ids))(*device_ids)
                rc = lib.axon_start_nrt_profile(ids, len(device_ids))
            else:
                rc = lib.axon_start_nrt_profile(None, 0)
            if rc != 0:
                raise RuntimeError(f"axon_start_nrt_profile rc={rc}")
            try:
                yield
            finally:
                lib.axon_stop_nrt_profile(str(output_dir).encode())

        mod = types.ModuleType("antenv.axon_hooks")
        mod.get_axon_ntff_profile_hook = lambda: _hook
        mod.set_axon_ntff_profile_hook = lambda h: None
        sys.modules["antenv.axon_hooks"] = mod
        return True
    except Exception:
        return False


def _smin(kc):
    """First slot whose row block attends key chunk kc."""
    return 0 if kc <= 4 else (kc - 1) // 4


def _emit(nc):
    xkT_d = nc.declare_dram_parameter("xkT", [D, 2048], BF16, isOutput=False)
    xqTb_d = nc.declare_dram_parameter("xqTb", [D, R], BF16, isOutput=False)
    wq_d = nc.declare_dram_parameter("wq", [D, D], BF16, isOutput=False)
    wkv_d = nc.declare_dram_parameter("wkv", [D, 2 * DH], BF16, isOutput=False)
    wout_d = nc.declare_dram_parameter("wout", [D, D], BF16, isOutput=False)
    thr_d = nc.declare_dram_parameter("thresh", [R], F32, isOutput=False)
    comb_d = nc.declare_dram_parameter("comb128", [P], F32, isOutput=False)
    nkT_d = nc.declare_dram_parameter("nullkT", [DH, 2], BF16, isOutput=False)
    nv_d = nc.declare_dram_parameter("nullv", [2, DH], BF16, isOutput=False)
    iota_d = nc.declare_dram_parameter("iota", [P], F32, isOutput=False)
    out_d = nc.declare_dram_parameter("out", [R, D], F32, isOutput=True)

    def bcast(ap, n):
        return bass.AP(tensor=ap.tensor, offset=ap.offset,
                       ap=[[0, n]] + [list(x) for x in ap.ap])

    with ExitStack() as ctx:
        tc = ctx.enter_context(tile.TileContext(nc))
        singles = ctx.enter_context(tc.tile_pool(name="singles", bufs=1))
        work = ctx.enter_context(tc.tile_pool(name="work", bufs=2))
        small = ctx.enter_context(tc.tile_pool(name="small", bufs=4))
        expp = ctx.enter_context(tc.tile_pool(name="expp", bufs=3))
        outp = ctx.enter_context(tc.tile_pool(name="outp", bufs=2))
        # PSUM: pool_big slots are 2 banks ([128,2,512] f32); pool_oT 1 bank.
        pool_big = ctx.enter_context(tc.tile_pool(name="pbig", bufs=2, space="PSUM"))
        pool_oT = ctx.enter_context(tc.tile_pool(name="poT", bufs=4, space="PSUM"))
        dramp = ctx.enter_context(tc.tile_pool(name="dram", bufs=2, space="DRAM"))

        # ---------- constants & weights ----------
        # DMA order matters: xqTb first (mean matmuls), then wq (q proj), wkv,
        # xkT bulk, wout last (only needed at the end).
        wq_sb = singles.tile([P, 8, D], BF16)
        wout_sb = singles.tile([P, 8, D], BF16)
        wkv_sb = singles.tile([P, 8, 2 * DH], BF16)
        xqTb = singles.tile([P, 8, R], BF16)
        xkT_sb = singles.tile([P, 8, 2048], BF16)
        for o in range(8):
            nc.sync.dma_start(out=xqTb[:, o, :], in_=xqTb_d[o * P:(o + 1) * P, :])
        for o in range(8):
            nc.sync.dma_start(out=wq_sb[:, o, :], in_=wq_d[o * P:(o + 1) * P, :])
            nc.sync.dma_start(out=wkv_sb[:, o, :], in_=wkv_d[o * P:(o + 1) * P, :])
        for o in range(8):
            nc.sync.dma_start(out=xkT_sb[:, o, :], in_=xkT_d[o * P:(o + 1) * P, :])
        for o in range(8):
            nc.sync.dma_start(out=wout_sb[:, o, :], in_=wout_d[o * P:(o + 1) * P, :])
        comb_sb = singles.tile([P, 1], F32)
        nc.scalar.dma_start(out=comb_sb, in_=comb_d[:].rearrange("(p o) -> p o", o=1))
        thr_sb = singles.tile([P, R], F32)
        nc.scalar.dma_start(out=thr_sb, in_=bcast(thr_d[:], P))
        iota_sb = singles.tile([P, 1], F32)
        nc.scalar.dma_start(out=iota_sb, in_=iota_d[:].rearrange("(p o) -> p o", o=1))
        jcols = singles.tile([P, H], F32)
        for kc in range(1, NCH):
            nc.vector.tensor_scalar_add(jcols[:, kc - 1:kc], iota_sb, float(kc * P))
        eps_sb = singles.tile([P, 1], F32)
        nc.vector.memset(eps_sb, 1e-24)
        ones_sb = singles.tile([P, 1], BF16)
        nc.vector.memset(ones_sb, 1.0)
        # [128, 2] parity selector: col 0 = partitions 0:64, col 1 = 64:128
        sel2 = singles.tile([P, 2], BF16)
        nc.vector.memset(sel2, 0.0)
        nc.vector.memset(sel2[0:DH, 0:1], 1.0)
        nc.vector.memset(sel2[DH:P, 1:2], 1.0)
        # exp bias for chunk 0: 0 on the two null-key partitions, -1e4 on the
        # 126 dead padding partitions (exp -> 0, no mask multiply needed)
        nullb = singles.tile([P, 1], F32)
        nc.vector.tensor_scalar(nullb, iota_sb, 1.0, 1.0, AL.subtract, AL.min)
        nc.vector.tensor_scalar(nullb, nullb, 0.0, -1e4, AL.max, AL.mult)
        zerob = singles.tile([P, 1], F32)
        nc.vector.memset(zerob, 0.0)
        # causal diag masks, one [key, row-of-diag-slot] 0/1 tile per chunk
        mk_all = singles.tile([P, H, P], BF16)
        for kc in range(1, NCH):
            s = (kc - 1) // 4
            m = mk_all[:, kc - 1, :]
            nc.vector.tensor_scalar(m, thr_sb[:, s * P:(s + 1) * P],
                                    jcols[:, kc - 1:kc], None, AL.subtract)
            nc.vector.tensor_scalar(m, m, 1.0, 0.0, AL.min, AL.max)

        # ---------- Q^T path ----------
        # row means via ones-matmul
        mu_ps = pool_oT.tile([1, R], F32, tag="oT")
        for dci in range(8):
            nc.tensor.matmul(mu_ps, lhsT=ones_sb, rhs=xqTb[:, dci, :],
                             start=(dci == 0), stop=(dci == 7))
        nmu = small.tile([1, R], BF16, tag="nmu")
        nc.vector.tensor_scalar_mul(nmu, mu_ps, -1.0 / D)
        d_nmu = dramp.tile([1, R], BF16, tag="dnmu")
        nc.scalar.dma_start(out=d_nmu, in_=nmu)
        mu_b = singles.tile([P, R], BF16)
        nc.scalar.dma_start(out=mu_b, in_=bcast(d_nmu[0, :], P))
        # mean-center in place (rstd cancels in the per-head l2norm);
        # per-dci so the first q matmul unblocks as soon as slice 0 is ready
        for dci in range(8):
            nc.vector.tensor_tensor(xqTb[:, dci, :], xqTb[:, dci, :],
                                    mu_b, AL.add)
        # q^T per head pair, raw (pre-norm)
        qt_sb = singles.tile([P, 8, R], BF16)
        for j in range(8):
            q_ps = pool_big.tile([P, R], F32, tag="big")
            for dci in range(8):
                nc.tensor.matmul(q_ps, lhsT=wq_sb[:, dci, j * P:(j + 1) * P],
                                 rhs=xqTb[:, dci, :],
                                 start=(dci == 0), stop=(dci == 7))
            nc.vector.tensor_copy(out=qt_sb[:, j, :], in_=q_ps)

        # ---------- q l2norm (partition-dim sumsq via sel2-matmuls) ----------
        # split into two independent half-chains (pairs 0-3 / 4-7) so the
        # first half's rq broadcasts land while the second half still bounces
        qsq = singles.tile([P, 8, R], BF16)
        rq_b = singles.tile([P, 8, R], BF16)
        for hf in range(2):
            hps = range(4 * hf, 4 * hf + 4)
            nc.vector.tensor_mul(qsq[:, 4 * hf:4 * hf + 4, :],
                                 qt_sb[:, 4 * hf:4 * hf + 4, :],
                                 qt_sb[:, 4 * hf:4 * hf + 4, :])
            qn_st = singles.tile([2, 4, R], F32, name=f"qnst_{hf}")
            for hp in hps:
                nq_ps = pool_oT.tile([2, R], F32, tag="oT", name=f"nq_{hp}")
                nc.tensor.matmul(nq_ps, lhsT=sel2, rhs=qsq[:, hp, :],
                                 start=True, stop=True)
                nc.vector.tensor_copy(out=qn_st[:, hp - 4 * hf, :], in_=nq_ps)
            d_qn = dramp.tile([2, 4, R], F32, tag=f"dqn{hf}")
            nc.scalar.dma_start(out=d_qn, in_=qn_st)
            qn2 = small.tile([DH, DH], F32, tag="qn2")
            nc.scalar.dma_start(
                out=qn2,
                in_=d_qn[:, :, :].rearrange("a b (p f) -> (a b p) f", f=DH))
            qm2 = small.tile([DH, DH], F32, tag="qm2")
            nc.scalar.activation(out=qm2, in_=qn2, func=AF.Sqrt,
                                 bias=eps_sb[0:DH, :])
            rq2 = small.tile([DH, DH], BF16, tag="rq2")
            with nc.allow_low_precision(reason="rq feeds bf16 matmul"):
                nc.vector.reciprocal(out=rq2, in_=qm2)
            d_rq = dramp.tile([DH, DH], BF16, tag=f"drq{hf}")
            nc.scalar.dma_start(out=d_rq, in_=rq2)
            if hf == 1:
                # exp-table prefetch once all sqrt work is emitted
                dummy = small.tile([1, 1], F32, tag="dummy")
                nc.scalar.activation(out=dummy, in_=eps_sb[0:1, :], func=AF.Exp)
            d_rq_flat = d_rq[:, :].rearrange("p f -> (p f)")
            for hp in hps:
                for par in range(2):
                    off = (par * 4 + (hp - 4 * hf)) * R
                    nc.scalar.dma_start(out=rq_b[par * DH:(par + 1) * DH, hp, :],
                                        in_=bcast(d_rq_flat[off:off + R], DH))
                nc.vector.tensor_scalar_mul(rq_b[:, hp, :], rq_b[:, hp, :], comb_sb)
                nc.vector.tensor_mul(qt_sb[:, hp, :], qt_sb[:, hp, :], rq_b[:, hp, :])

        # ---------- K^T, V from full batch row set ----------
        # chunk 0 (host-normalized nulls + zero pads) is complete right away,
        # so chunk-0 scores never wait on the k-norm chain
        kt_sb = singles.tile([P, NKEY], BF16)
        vt_sb = singles.tile([P, 2048], BF16)
        nc.vector.memset(kt_sb[0:DH, 0:P], 0.0)
        nc.scalar.dma_start(out=kt_sb[0:DH, 0:2], in_=nkT_d[:])
        nc.scalar.dma_start(out=kt_sb[DH:P, 0:P], in_=kt_sb[0:DH, 0:P])
        for kb in range(4):
            pkv = pool_big.tile([P, 512], F32, tag="big")
            for dci in range(8):
                nc.tensor.matmul(pkv, lhsT=wkv_sb[:, dci, :],
                                 rhs=xkT_sb[:, dci, kb * 512:(kb + 1) * 512],
                                 start=(dci == 0), stop=(dci == 7))
            nc.vector.tensor_copy(out=kt_sb[0:DH, P + kb * 512:P + (kb + 1) * 512],
                                  in_=pkv[0:DH, :])
            nc.vector.tensor_copy(out=vt_sb[0:DH, kb * 512:(kb + 1) * 512],
                                  in_=pkv[DH:2 * DH, :])
        # k l2norm is folded into the exp: scores psc partitions ARE keys, so
        # exp(rk[key]*score) via the activation's per-partition scale AP. kt_sb
        # stays raw (scores are scale-invariant per key), removing the rk
        # broadcast + normalize TTs from the attention critical path.
        # rkc[p, kc] = 1/||k_{kc*128+p}|| from a chunk-major [128,17] reshape.
        ksq = singles.tile([P, NKEY], BF16)
        nc.vector.tensor_mul(ksq[0:DH, :], kt_sb[0:DH, :], kt_sb[0:DH, :])
        ks_st = singles.tile([1, NKEY], F32)
        for i in range(5):
            w = 512 if i < 4 else 128
            kss = pool_oT.tile([1, 512], F32, tag="oT", name=f"kss_{i}")
            nc.tensor.matmul(kss[0:1, 0:w], lhsT=ones_sb[0:DH, :],
                             rhs=ksq[0:DH, i * 512:i * 512 + w],
                             start=True, stop=True)
            nc.vector.tensor_copy(out=ks_st[0:1, i * 512:i * 512 + w],
                                  in_=kss[0:1, 0:w])
        d_ks = dramp.tile([1, NKEY], F32, tag="dks")
        nc.scalar.dma_start(out=d_ks, in_=ks_st)
        ks2 = small.tile([P, NCH], F32, tag="ks2")
        nc.scalar.dma_start(out=ks2, in_=d_ks[0, :].rearrange("(f p) -> p f", p=P))
        kn2 = small.tile([P, NCH], F32, tag="kn2")
        nc.scalar.activation(out=kn2, in_=ks2, func=AF.Sqrt, bias=eps_sb)
        rkc = singles.tile([P, NCH], F32)
        nc.vector.reciprocal(out=rkc, in_=kn2)
        # duplicate raw x-key cols to partitions 64:128 (no norm dependency)
        for i in range(4):
            sl = slice(P + i * 512, P + (i + 1) * 512)
            nc.sync.dma_start(out=kt_sb[DH:P, sl], in_=kt_sb[0:DH, sl])
        # V row-major per chunk via PE transpose (bf16 identity) + ones column
        ident_bf = singles.tile([P, P], BF16)
        make_identity(nc, ident_bf)
        v_sb = singles.tile([P, NCH, DH + 1], BF16)
        nc.vector.memset(v_sb[:, 0, 0:DH], 0.0)
        nc.vector.memset(v_sb[:, :, DH:DH + 1], 1.0)
        nc.scalar.dma_start(out=v_sb[0:2, 0, 0:DH], in_=nv_d[:])
        for ch in range(1, NCH):
            pt_v = pool_big.tile([P, DH], BF16, tag="big")
            nc.tensor.transpose(pt_v, vt_sb[0:DH, (ch - 1) * P:ch * P],
                                ident_bf[0:DH, 0:DH])
            nc.vector.tensor_copy(out=v_sb[:, ch, 0:DH], in_=pt_v)

        # ---------- attention, head pair by head pair ----------
        # Chunk loop is software-pipelined: scores/exp/mask for chunk kc are
        # emitted one step ahead of attn@V for chunk kc-1 so the in-order PE
        # stream always has a ready matmul. Masks alternate DVE/GpSimd.
        oT_sb = singles.tile([P, 8, R], BF16)
        for hp in range(8):
            oT = {par: pool_oT.tile([DH + 1, R], F32, tag="oT",
                                    name=f"oT_{hp}_{par}")
                  for par in range(2)}
            es_q = {}
            for kc in range(NCH + 1):
                if kc < NCH:
                    sm = _smin(kc)
                    nv = R - sm * P
                    psc = pool_big.tile([P, 2, R], F32, tag="big")
                    for par in range(2):
                        nc.tensor.matmul(psc[:, par, 0:nv],
                                         lhsT=kt_sb[par * DH:(par + 1) * DH,
                                                    kc * P:(kc + 1) * P],
                                         rhs=qt_sb[par * DH:(par + 1) * DH, hp,
                                                   sm * P:R],
                                         start=True, stop=True)
                    es = expp.tile([P, 2, R], BF16, tag="es")
                    nc.scalar.activation(out=es[:, :, 0:nv], in_=psc[:, :, 0:nv],
                                         func=AF.Exp, scale=rkc[:, kc:kc + 1],
                                         bias=(nullb if kc == 0 else zerob))
                    if kc > 0:
                        meng = nc.gpsimd if (kc % 2) else nc.vector
                        for par in range(2):
                            meng.tensor_mul(es[:, par, 0:P], es[:, par, 0:P],
                                            mk_all[:, kc - 1, :])
                    es_q[kc] = es
                if kc >= 1:
                    pv = kc - 1
                    smv = _smin(pv)
                    nvv = R - smv * P
                    for par in range(2):
                        nc.tensor.matmul(oT[par][:, smv * P:R],
                                         lhsT=v_sb[:, pv, :],
                                         rhs=es_q[pv][:, par, 0:nvv],
                                         start=(pv == 0), stop=(pv == NCH - 1))
                    del es_q[pv]
            # denominators: stage the two [1,R] rows, bounce via DRAM into a
            # [128,8] reshape for one cheap reciprocal, broadcast back.
            den_st = work.tile([P, 2, R], F32, tag="denst")
            for par in range(2):
                nc.vector.tensor_copy(out=den_st[DH:DH + 1, par, :],
                                      in_=oT[par][DH:DH + 1, :])
            d_den = dramp.tile([2, R], F32, tag="dden")
            nc.sync.dma_start(out=d_den, in_=den_st[DH:DH + 1, :, :])
            den2 = small.tile([P, 8], F32, tag="den2")
            nc.sync.dma_start(out=den2,
                              in_=d_den[:, :].rearrange("a (p f) -> (a p) f", f=8))
            rr2 = small.tile([P, 8], F32, tag="rr2")
            nc.vector.reciprocal(out=rr2, in_=den2)
            d_rr = dramp.tile([P, 8], F32, tag="drr")
            nc.sync.dma_start(out=d_rr, in_=rr2)
            d_rr_flat = d_rr[:, :].rearrange("p f -> (p f)")
            for par in range(2):
                rr_b = work.tile([DH, R], F32, tag="rrb")
                nc.sync.dma_start(out=rr_b,
                                  in_=bcast(d_rr_flat[par * R:(par + 1) * R], DH))
                nc.vector.tensor_tensor(oT_sb[par * DH:(par + 1) * DH, hp, :],
                                        oT[par][0:DH, :], rr_b, AL.mult)

        # ---------- output projection (two waves) ----------
        # Wave 1 accumulates pairs 0..5 into SBUF staging as soon as they are
        # normalized; wave 2 adds pairs 6..7 on the DVE and ships one regular
        # DMA per block -- no slow SWDGE accumulate-DMAs in the kernel tail.
        ob1 = singles.tile([P, NB, 2, 512], F32)
        for s in range(NB):
            for nh in range(2):
                pf = pool_big.tile([P, 512], F32, tag="big")
                for hp in range(6):
                    nc.tensor.matmul(pf, lhsT=oT_sb[:, hp, s * P:(s + 1) * P],
                                     rhs=wout_sb[:, hp, nh * 512:(nh + 1) * 512],
                                     start=(hp == 0), stop=(hp == 5))
                nc.vector.tensor_copy(out=ob1[:, s, nh, :], in_=pf)
        for s in range(NB):
            for nh in range(2):
                pf = pool_big.tile([P, 512], F32, tag="big")
                for hp in range(6, 8):
                    nc.tensor.matmul(pf, lhsT=oT_sb[:, hp, s * P:(s + 1) * P],
                                     rhs=wout_sb[:, hp, nh * 512:(nh + 1) * 512],
                                     start=(hp == 6), stop=(hp == 7))
                ob = outp.tile([P, 512], F32, tag="ob")
                nc.vector.tensor_tensor(ob, pf, ob1[:, s, nh, :], AL.add)
                nc.sync.dma_start(out=out_d[s * P:(s + 1) * P, nh * 512:(nh + 1) * 512],
                                  in_=ob)
    return nc


def _get_nc():
    if "nc" not in _CACHE:
        nc = bacc.Bacc(None, target_bir_lowering=False)
        _emit(nc)
        nc.finalize()
        _CACHE["nc"] = nc
    return _CACHE["nc"]


def kernel(x, gamma, Wq, Wkv, q_scale, k_scale, null_kv, Wout):
    import ml_dtypes
    bf16 = ml_dtypes.bfloat16

    x = np.asarray(x, np.float32)
    gamma = np.asarray(gamma, np.float32)
    Wq = np.asarray(Wq, np.float32)
    Wkv = np.asarray(Wkv, np.float32)
    q_scale = np.asarray(q_scale, np.float32)
    k_scale = np.asarray(k_scale, np.float32)
    null_kv = np.asarray(null_kv, np.float32)
    Wout = np.asarray(Wout, np.float32)
    b, n, d = x.shape

    wq_eff = np.ascontiguousarray((gamma[:, None] * Wq).astype(bf16))
    wkv_bf = np.ascontiguousarray(Wkv.astype(bf16))
    wout_bf = np.ascontiguousarray(Wout.astype(bf16))
    comb128 = np.ascontiguousarray(np.tile(q_scale * k_scale * 8.0, 2))
    iota = np.arange(P, dtype=np.float32)
    # null keys pre-normalized on host: chunk 0 of the score matmuls then has
    # no dependency on the on-chip k-norm chain (rk multiply is a no-op there)
    nk = null_kv[0]
    nk = nk / np.maximum(np.linalg.norm(nk, axis=1, keepdims=True), 1e-12)
    nullkT = np.ascontiguousarray(nk.T.astype(bf16))
    nullv = np.ascontiguousarray(null_kv[1].astype(bf16))
    xkT_bf = [np.ascontiguousarray(x[bi].T.astype(bf16)) for bi in range(b)]

    in_maps = []
    row_sets = []
    for c in range(8):
        bi, qc = c // 4, c % 4
        blocks = [qc, 4 + qc, 8 + qc, 12 + qc]
        rows = np.concatenate([np.arange(P * t, P * t + P) for t in blocks])
        row_sets.append((bi, rows))
        thresh = np.where(rows < 64, 66, rows + 3).astype(np.float32) + 126.0
        in_maps.append({
            "xkT": xkT_bf[bi],
            "xqTb": np.ascontiguousarray(x[bi][rows].T.astype(bf16)),
            "wq": wq_eff,
            "wkv": wkv_bf,
            "wout": wout_bf,
            "thresh": thresh,
            "comb128": comb128,
            "nullkT": nullkT,
            "nullv": nullv,
            "iota": iota,
        })

    nc = _get_nc()
    # Tracing (for the HW-exec-time print) only when explicitly requested:
    # it adds substantial host-side post-processing to each run.
    import os
    trace = bool(os.environ.get("BASS_KERNEL_TRACE")) and _install_ntff_hook()
    try:
        res = run_bass_kernel_spmd(nc, in_maps, core_ids=list(range(8)), trace=trace)
    except (ImportError, ModuleNotFoundError):
        res = run_bass_kernel_spmd(nc, in_maps, core_ids=list(range(8)), trace=False)
    if getattr(res, "exec_time_ns", None) is not None:
        print(f"HW exec time: {res.exec_time_ns} ns")
    out = np.empty((b, n, d), dtype=np.float32)
    for c in range(8):
        bi, rows = row_sets[c]
        out[bi][rows] = res.results[c]["out"]
    return out


# revision 47
# speedup vs baseline: 1.1714x; 1.1714x over previous
"""Distributed Trainium2 kernel for nn_Attention_81028853007052.

8 cores = batch(2) x 4 query-block groups. Core (b, qc) processes the four
interleaved 128-row query blocks {qc, 4+qc, 8+qc, 12+qc} of batch b; slot s
(local block s, global block 4s+qc) attends keys [0, 512(s+1)+2) -- causally
balanced and SPMD-uniform. Per-row causal thresholds are passed as data.

Internal key layout: col 0,1 = null kv; cols 2..127 dead padding; col 128+j =
x-key j (ref col j+2). thresh' = ref_thresh + 126 compares directly against
internal col index.

Everything is computed in transposed ("T") layouts so no PE transposes are
needed and every matmul has a wide (>=256) moving dim:
  - qT = Wq^T @ (x - mu)^T directly from host-transposed x; the layernorm
    rstd cancels inside the per-head l2norm so only the mean is subtracted.
  - l2norm partition-dim sums of squares via ones-vector matmuls.
  - scores^T[key, row] accumulated per 128-key chunk; even/odd head pairs
    issued back-to-back at partition bases 0/64 for row-tile concurrency.
  - exp on ScalarE with a per-partition bias that kills the dead padding
    keys of chunk 0; causal diag masks (0/1, data-driven thresholds) are
    multiplied on GpSimd which is otherwise idle.
  - attn@V flipped: out^T[c, row] = V^T @ es accumulated over chunks, with
    a ones-column in V giving the softmax denominator.
  - out^T feeds the output projection directly (it is already the lhsT).
All matmuls run bf16 (1 cycle/row); accumulation is fp32 in PSUM.
"""

import numpy as np
from contextlib import ExitStack

import concourse.bass as bass
import concourse.mybir as mybir
import concourse.tile as tile
from concourse import bacc
from concourse.bass_utils import run_bass_kernel_spmd
from concourse.masks import make_identity

P = 128
D = 1024
H = 16
DH = 64
R = 512          # query rows per core
NB = 4           # local query blocks (= slots)
NCH = 17         # key chunks of 128 (1 null/pad chunk + 16 x chunks)
NKEY = NCH * P   # 2176
F32 = mybir.dt.float32
BF16 = mybir.dt.bfloat16
AF = mybir.ActivationFunctionType
AL = mybir.AluOpType
X = mybir.AxisListType.X

_CACHE = {}


def _install_ntff_hook():
    """Best-effort: register the axon NTFF profile hook so trace=True works."""
    import sys
    if "antenv.axon_hooks" in sys.modules:
        return True
    try:
        import contextlib
        import ctypes
        import types

        lib = ctypes.CDLL("/opt/axon/libaxon_pjrt.so")
        if not hasattr(lib, "axon_start_nrt_profile"):
            return False
        lib.axon_start_nrt_profile.argtypes = [
            ctypes.POINTER(ctypes.c_int64), ctypes.c_size_t]
        lib.axon_start_nrt_profile.restype = ctypes.c_int64
        lib.axon_stop_nrt_profile.argtypes = [ctypes.c_char_p]
        lib.axon_stop_nrt_profile.restype = ctypes.c_int64

        @contextlib.contextmanager
        def _hook(output_dir, device_ids):
            import jax
            jax.devices()
            if device_ids:
                ids = (ctypes.c_int64 * len(device_ids))(*device_ids)
                rc = lib.axon_start_nrt_profile(ids, len(device_ids))
            else:
                rc = lib.axon_start_nrt_profile(None, 0)
            if rc != 0:
                raise RuntimeError(f"axon_start_nrt_profile rc={rc}")
            try:
                yield
            finally:
                lib.axon_stop_nrt_profile(str(output_dir).encode())

        mod = types.ModuleType("antenv.axon_hooks")
        mod.get_axon_ntff_profile_hook = lambda: _hook
        mod.set_axon_ntff_profile_hook = lambda h: None
        sys.modules["antenv.axon_hooks"] = mod
        return True
    except Exception:
        return False


def _smin(kc):
    """First slot whose row block attends key chunk kc."""
    return 0 if kc <= 4 else (kc - 1) // 4


def _emit(nc):
    xkT_d = nc.declare_dram_parameter("xkT", [D, 2048], BF16, isOutput=False)
    xqTb_d = nc.declare_dram_parameter("xqTb", [D, R], BF16, isOutput=False)
    wq_d = nc.declare_dram_parameter("wq", [D, D], BF16, isOutput=False)
    wkv_d = nc.declare_dram_parameter("wkv", [D, 2 * DH], BF16, isOutput=False)
    wout_d = nc.declare_dram_parameter("wout", [D, D], BF16, isOutput=False)
    thr_d = nc.declare_dram_parameter("thresh", [R], F32, isOutput=False)
    comb_d = nc.declare_dram_parameter("comb128", [P], F32, isOutput=False)
    nkT_d = nc.declare_dram_parameter("nullkT", [DH, 2], BF16, isOutput=False)
    nv_d = nc.declare_dram_parameter("nullv", [2, DH], BF16, isOutput=False)
    iota_d = nc.declare_dram_parameter("iota", [P], F32, isOutput=False)
    out_d = nc.declare_dram_parameter("out", [R, D], F32, isOutput=True)

    def bcast(ap, n):
        return bass.AP(tensor=ap.tensor, offset=ap.offset,
                       ap=[[0, n]] + [list(x) for x in ap.ap])

    with ExitStack() as ctx:
        tc = ctx.enter_context(tile.TileContext(nc))
        singles = ctx.enter_context(tc.tile_pool(name="singles", bufs=1))
        work = ctx.enter_context(tc.tile_pool(name="work", bufs=2))
        small = ctx.enter_context(tc.tile_pool(name="small", bufs=4))
        expp = ctx.enter_context(tc.tile_pool(name="expp", bufs=3))
        outp = ctx.enter_context(tc.tile_pool(name="outp", bufs=2))
        # PSUM: pool_big slots are 2 banks ([128,2,512] f32); pool_oT 1 bank.
        pool_big = ctx.enter_context(tc.tile_pool(name="pbig", bufs=2, space="PSUM"))
        pool_oT = ctx.enter_context(tc.tile_pool(name="poT", bufs=4, space="PSUM"))
        dramp = ctx.enter_context(tc.tile_pool(name="dram", bufs=2, space="DRAM"))

        # ---------- constants & weights ----------
        # DMA order matters: xqTb first (mean matmuls), then wq (q proj), wkv,
        # xkT bulk, wout last (only needed at the end).
        wq_sb = singles.tile([P, 8, D], BF16)
        wout_sb = singles.tile([P, 8, D], BF16)
        wkv_sb = singles.tile([P, 8, 2 * DH], BF16)
        xqTb = singles.tile([P, 8, R], BF16)
        xkT_sb = singles.tile([P, 8, 2048], BF16)
        for o in range(8):
            nc.sync.dma_start(out=xqTb[:, o, :], in_=xqTb_d[o * P:(o + 1) * P, :])
        for o in range(8):
            nc.sync.dma_start(out=wq_sb[:, o, :], in_=wq_d[o * P:(o + 1) * P, :])
            nc.sync.dma_start(out=wkv_sb[:, o, :], in_=wkv_d[o * P:(o + 1) * P, :])
        for o in range(8):
            nc.sync.dma_start(out=xkT_sb[:, o, :], in_=xkT_d[o * P:(o + 1) * P, :])
        for o in range(8):
            nc.sync.dma_start(out=wout_sb[:, o, :], in_=wout_d[o * P:(o + 1) * P, :])
        comb_sb = singles.tile([P, 1], F32)
        nc.scalar.dma_start(out=comb_sb, in_=comb_d[:].rearrange("(p o) -> p o", o=1))
        thr_sb = singles.tile([P, R], F32)
        nc.scalar.dma_start(out=thr_sb, in_=bcast(thr_d[:], P))
        iota_sb = singles.tile([P, 1], F32)
        nc.scalar.dma_start(out=iota_sb, in_=iota_d[:].rearrange("(p o) -> p o", o=1))
        jcols = singles.tile([P, H], F32)
        for kc in range(1, NCH):
            nc.vector.tensor_scalar_add(jcols[:, kc - 1:kc], iota_sb, float(kc * P))
        eps_sb = singles.tile([P, 1], F32)
        nc.vector.memset(eps_sb, 1e-24)
        ones_sb = singles.tile([P, 1], BF16)
        nc.vector.memset(ones_sb, 1.0)
        # [128, 2] parity selector: col 0 = partitions 0:64, col 1 = 64:128
        sel2 = singles.tile([P, 2], BF16)
        nc.vector.memset(sel2, 0.0)
        nc.vector.memset(sel2[0:DH, 0:1], 1.0)
        nc.vector.memset(sel2[DH:P, 1:2], 1.0)
        # exp bias for chunk 0: 0 on the two null-key partitions, -1e4 on the
        # 126 dead padding partitions (exp -> 0, no mask multiply needed)
        nullb = singles.tile([P, 1], F32)
        nc.vector.tensor_scalar(nullb, iota_sb, 1.0, 1.0, AL.subtract, AL.min)
        nc.vector.tensor_scalar(nullb, nullb, 0.0, -1e4, AL.max, AL.mult)
        zerob = singles.tile([P, 1], F32)
        nc.vector.memset(zerob, 0.0)
        # causal diag masks, one [key, row-of-diag-slot] 0/1 tile per chunk
        mk_all = singles.tile([P, H, P], BF16)
        for kc in range(1, NCH):
            s = (kc - 1) // 4
            m = mk_all[:, kc - 1, :]
            nc.vector.tensor_scalar(m, thr_sb[:, s * P:(s + 1) * P],
                                    jcols[:, kc - 1:kc], None, AL.subtract)
            nc.vector.tensor_scalar(m, m, 1.0, 0.0, AL.min, AL.max)

        # ---------- Q^T path ----------
        # row means via ones-matmul
        mu_ps = pool_oT.tile([1, R], F32, tag="oT")
        for dci in range(8):
            nc.tensor.matmul(mu_ps, lhsT=ones_sb, rhs=xqTb[:, dci, :],
                             start=(dci == 0), stop=(dci == 7))
        nmu = small.tile([1, R], BF16, tag="nmu")
        nc.vector.tensor_scalar_mul(nmu, mu_ps, -1.0 / D)
        d_nmu = dramp.tile([1, R], BF16, tag="dnmu")
        nc.scalar.dma_start(out=d_nmu, in_=nmu)
        mu_b = singles.tile([P, R], BF16)
        nc.scalar.dma_start(out=mu_b, in_=bcast(d_nmu[0, :], P))
        # mean-center in place (rstd cancels in the per-head l2norm);
        # per-dci so the first q matmul unblocks as soon as slice 0 is ready
        for dci in range(8):
            nc.vector.tensor_tensor(xqTb[:, dci, :], xqTb[:, dci, :],
                                    mu_b, AL.add)
        # q^T per head pair, raw (pre-norm)
        qt_sb = singles.tile([P, 8, R], BF16)
        for j in range(8):
            q_ps = pool_big.tile([P, R], F32, tag="big")
            for dci in range(8):
                nc.tensor.matmul(q_ps, lhsT=wq_sb[:, dci, j * P:(j + 1) * P],
                                 rhs=xqTb[:, dci, :],
                                 start=(dci == 0), stop=(dci == 7))
            nc.vector.tensor_copy(out=qt_sb[:, j, :], in_=q_ps)

        # ---------- q l2norm (partition-dim sumsq via sel2-matmuls) ----------
        # sumsq rows staged to [2,8,512], bounced to a [128,64] reshape for one
        # wide sqrt+recip, then broadcast back per head.
        qsq = singles.tile([P, 8, R], BF16)
        nc.vector.tensor_mul(qsq, qt_sb, qt_sb)
        qn_st = singles.tile([2, 8, R], F32)
        for hp in range(8):
            nq_ps = pool_oT.tile([2, R], F32, tag="oT", name=f"nq_{hp}")
            nc.tensor.matmul(nq_ps, lhsT=sel2, rhs=qsq[:, hp, :],
                             start=True, stop=True)
            nc.vector.tensor_copy(out=qn_st[:, hp, :], in_=nq_ps)
        d_qn = dramp.tile([2, 8, R], F32, tag="dqn")
        nc.scalar.dma_start(out=d_qn, in_=qn_st)
        qn2 = small.tile([P, DH], F32, tag="qn2")
        nc.scalar.dma_start(out=qn2,
                          in_=d_qn[:, :, :].rearrange("a b (p f) -> (a b p) f", f=DH))
        qm2 = small.tile([P, DH], F32, tag="qm2")
        nc.scalar.activation(out=qm2, in_=qn2, func=AF.Sqrt, bias=eps_sb)
        rq2 = small.tile([P, DH], BF16, tag="rq2")
        with nc.allow_low_precision(reason="rq feeds bf16 matmul"):
            nc.vector.reciprocal(out=rq2, in_=qm2)
        d_rq = dramp.tile([P, DH], BF16, tag="drq")
        nc.scalar.dma_start(out=d_rq, in_=rq2)
        # exp-table prefetch: all sqrt work is done, load the exp set now
        dummy = small.tile([1, 1], F32, tag="dummy")
        nc.scalar.activation(out=dummy, in_=eps_sb[0:1, :], func=AF.Exp)
        rq_b = singles.tile([P, 8, R], BF16)
        d_rq_flat = d_rq[:, :].rearrange("p f -> (p f)")
        # per-pair: broadcast rq, fold comb (q_scale*k_scale*8), scale q
        for hp in range(8):
            for par in range(2):
                off = (par * 8 + hp) * R
                nc.scalar.dma_start(out=rq_b[par * DH:(par + 1) * DH, hp, :],
                                  in_=bcast(d_rq_flat[off:off + R], DH))
            nc.vector.tensor_scalar_mul(rq_b[:, hp, :], rq_b[:, hp, :], comb_sb)
            nc.vector.tensor_mul(qt_sb[:, hp, :], qt_sb[:, hp, :], rq_b[:, hp, :])

        # ---------- K^T, V from full batch row set ----------
        # chunk 0 (host-normalized nulls + zero pads) is complete right away,
        # so chunk-0 scores never wait on the k-norm chain
        kt_sb = singles.tile([P, NKEY], BF16)
        vt_sb = singles.tile([P, 2048], BF16)
        nc.vector.memset(kt_sb[0:DH, 0:P], 0.0)
        nc.scalar.dma_start(out=kt_sb[0:DH, 0:2], in_=nkT_d[:])
        nc.scalar.dma_start(out=kt_sb[DH:P, 0:P], in_=kt_sb[0:DH, 0:P])
        for kb in range(4):
            pkv = pool_big.tile([P, 512], F32, tag="big")
            for dci in range(8):
                nc.tensor.matmul(pkv, lhsT=wkv_sb[:, dci, :],
                                 rhs=xkT_sb[:, dci, kb * 512:(kb + 1) * 512],
                                 start=(dci == 0), stop=(dci == 7))
            nc.vector.tensor_copy(out=kt_sb[0:DH, P + kb * 512:P + (kb + 1) * 512],
                                  in_=pkv[0:DH, :])
            nc.vector.tensor_copy(out=vt_sb[0:DH, kb * 512:(kb + 1) * 512],
                                  in_=pkv[DH:2 * DH, :])
        # k l2norm is folded into the exp: scores psc partitions ARE keys, so
        # exp(rk[key]*score) via the activation's per-partition scale AP. kt_sb
        # stays raw (scores are scale-invariant per key), removing the rk
        # broadcast + normalize TTs from the attention critical path.
        # rkc[p, kc] = 1/||k_{kc*128+p}|| from a chunk-major [128,17] reshape.
        ksq = singles.tile([P, NKEY], BF16)
        nc.vector.tensor_mul(ksq[0:DH, :], kt_sb[0:DH, :], kt_sb[0:DH, :])
        ks_st = singles.tile([1, NKEY], F32)
        for i in range(5):
            w = 512 if i < 4 else 128
            kss = pool_oT.tile([1, 512], F32, tag="oT", name=f"kss_{i}")
            nc.tensor.matmul(kss[0:1, 0:w], lhsT=ones_sb[0:DH, :],
                             rhs=ksq[0:DH, i * 512:i * 512 + w],
                             start=True, stop=True)
            nc.vector.tensor_copy(out=ks_st[0:1, i * 512:i * 512 + w],
                                  in_=kss[0:1, 0:w])
        d_ks = dramp.tile([1, NKEY], F32, tag="dks")
        nc.scalar.dma_start(out=d_ks, in_=ks_st)
        ks2 = small.tile([P, NCH], F32, tag="ks2")
        nc.scalar.dma_start(out=ks2, in_=d_ks[0, :].rearrange("(f p) -> p f", p=P))
        kn2 = small.tile([P, NCH], F32, tag="kn2")
        nc.scalar.activation(out=kn2, in_=ks2, func=AF.Sqrt, bias=eps_sb)
        rkc = singles.tile([P, NCH], F32)
        nc.vector.reciprocal(out=rkc, in_=kn2)
        # duplicate raw x-key cols to partitions 64:128 (no norm dependency)
        for i in range(4):
            sl = slice(P + i * 512, P + (i + 1) * 512)
            nc.sync.dma_start(out=kt_sb[DH:P, sl], in_=kt_sb[0:DH, sl])
        # V row-major per chunk via PE transpose (bf16 identity) + ones column
        ident_bf = singles.tile([P, P], BF16)
        make_identity(nc, ident_bf)
        v_sb = singles.tile([P, NCH, DH + 1], BF16)
        nc.vector.memset(v_sb[:, 0, 0:DH], 0.0)
        nc.vector.memset(v_sb[:, :, DH:DH + 1], 1.0)
        nc.scalar.dma_start(out=v_sb[0:2, 0, 0:DH], in_=nv_d[:])
        for ch in range(1, NCH):
            pt_v = pool_big.tile([P, DH], BF16, tag="big")
            nc.tensor.transpose(pt_v, vt_sb[0:DH, (ch - 1) * P:ch * P],
                                ident_bf[0:DH, 0:DH])
            nc.vector.tensor_copy(out=v_sb[:, ch, 0:DH], in_=pt_v)

        # ---------- attention, head pair by head pair ----------
        # Chunk loop is software-pipelined: scores/exp/mask for chunk kc are
        # emitted one step ahead of attn@V for chunk kc-1 so the in-order PE
        # stream always has a ready matmul. Masks alternate DVE/GpSimd.
        oT_sb = singles.tile([P, 8, R], BF16)
        for hp in range(8):
            oT = {par: pool_oT.tile([DH + 1, R], F32, tag="oT",
                                    name=f"oT_{hp}_{par}")
                  for par in range(2)}
            es_q = {}
            for kc in range(NCH + 1):
                if kc < NCH:
                    sm = _smin(kc)
                    nv = R - sm * P
                    psc = pool_big.tile([P, 2, R], F32, tag="big")
                    for par in range(2):
                        nc.tensor.matmul(psc[:, par, 0:nv],
                                         lhsT=kt_sb[par * DH:(par + 1) * DH,
                                                    kc * P:(kc + 1) * P],
                                         rhs=qt_sb[par * DH:(par + 1) * DH, hp,
                                                   sm * P:R],
                                         start=True, stop=True)
                    es = expp.tile([P, 2, R], BF16, tag="es")
                    nc.scalar.activation(out=es[:, :, 0:nv], in_=psc[:, :, 0:nv],
                                         func=AF.Exp, scale=rkc[:, kc:kc + 1],
                                         bias=(nullb if kc == 0 else zerob))
                    if kc > 0:
                        meng = nc.gpsimd if (kc % 2) else nc.vector
                        for par in range(2):
                            meng.tensor_mul(es[:, par, 0:P], es[:, par, 0:P],
                                            mk_all[:, kc - 1, :])
                    es_q[kc] = es
                if kc >= 1:
                    pv = kc - 1
                    smv = _smin(pv)
                    nvv = R - smv * P
                    for par in range(2):
                        nc.tensor.matmul(oT[par][:, smv * P:R],
                                         lhsT=v_sb[:, pv, :],
                                         rhs=es_q[pv][:, par, 0:nvv],
                                         start=(pv == 0), stop=(pv == NCH - 1))
                    del es_q[pv]
            # denominators: stage the two [1,R] rows, bounce via DRAM into a
            # [128,8] reshape for one cheap reciprocal, broadcast back.
            den_st = work.tile([P, 2, R], F32, tag="denst")
            for par in range(2):
                nc.vector.tensor_copy(out=den_st[DH:DH + 1, par, :],
                                      in_=oT[par][DH:DH + 1, :])
            d_den = dramp.tile([2, R], F32, tag="dden")
            nc.sync.dma_start(out=d_den, in_=den_st[DH:DH + 1, :, :])
            den2 = small.tile([P, 8], F32, tag="den2")
            nc.sync.dma_start(out=den2,
                              in_=d_den[:, :].rearrange("a (p f) -> (a p) f", f=8))
            rr2 = small.tile([P, 8], F32, tag="rr2")
            nc.vector.reciprocal(out=rr2, in_=den2)
            d_rr = dramp.tile([P, 8], F32, tag="drr")
            nc.sync.dma_start(out=d_rr, in_=rr2)
            d_rr_flat = d_rr[:, :].rearrange("p f -> (p f)")
            for par in range(2):
                rr_b = work.tile([DH, R], F32, tag="rrb")
                nc.sync.dma_start(out=rr_b,
                                  in_=bcast(d_rr_flat[par * R:(par + 1) * R], DH))
                nc.vector.tensor_tensor(oT_sb[par * DH:(par + 1) * DH, hp, :],
                                        oT[par][0:DH, :], rr_b, AL.mult)

        # ---------- output projection (two waves) ----------
        # Wave 1 accumulates pairs 0..5 into SBUF staging as soon as they are
        # normalized; wave 2 adds pairs 6..7 on the DVE and ships one regular
        # DMA per block -- no slow SWDGE accumulate-DMAs in the kernel tail.
        ob1 = singles.tile([P, NB, 2, 512], F32)
        for s in range(NB):
            for nh in range(2):
                pf = pool_big.tile([P, 512], F32, tag="big")
                for hp in range(6):
                    nc.tensor.matmul(pf, lhsT=oT_sb[:, hp, s * P:(s + 1) * P],
                                     rhs=wout_sb[:, hp, nh * 512:(nh + 1) * 512],
                                     start=(hp == 0), stop=(hp == 5))
                nc.vector.tensor_copy(out=ob1[:, s, nh, :], in_=pf)
        for s in range(NB):
            for nh in range(2):
                pf = pool_big.tile([P, 512], F32, tag="big")
                for hp in range(6, 8):
                    nc.tensor.matmul(pf, lhsT=oT_sb[:, hp, s * P:(s + 1) * P],
                                     rhs=wout_sb[:, hp, nh * 512:(nh + 1) * 512],
                                     start=(hp == 6), stop=(hp == 7))
                ob = outp.tile([P, 512], F32, tag="ob")
                nc.vector.tensor_tensor(ob, pf, ob1[:, s, nh, :], AL.add)
                nc.sync.dma_start(out=out_d[s * P:(s + 1) * P, nh * 512:(nh + 1) * 512],
                                  in_=ob)
    return nc


def _get_nc():
    if "nc" not in _CACHE:
        nc = bacc.Bacc(None, target_bir_lowering=False)
        _emit(nc)
        nc.finalize()
        _CACHE["nc"] = nc
    return _CACHE["nc"]


def kernel(x, gamma, Wq, Wkv, q_scale, k_scale, null_kv, Wout):
    import ml_dtypes
    bf16 = ml_dtypes.bfloat16

    x = np.asarray(x, np.float32)
    gamma = np.asarray(gamma, np.float32)
    Wq = np.asarray(Wq, np.float32)
    Wkv = np.asarray(Wkv, np.float32)
    q_scale = np.asarray(q_scale, np.float32)
    k_scale = np.asarray(k_scale, np.float32)
    null_kv = np.asarray(null_kv, np.float32)
    Wout = np.asarray(Wout, np.float32)
    b, n, d = x.shape

    wq_eff = np.ascontiguousarray((gamma[:, None] * Wq).astype(bf16))
    wkv_bf = np.ascontiguousarray(Wkv.astype(bf16))
    wout_bf = np.ascontiguousarray(Wout.astype(bf16))
    comb128 = np.ascontiguousarray(np.tile(q_scale * k_scale * 8.0, 2))
    iota = np.arange(P, dtype=np.float32)
    # null keys pre-normalized on host: chunk 0 of the score matmuls then has
    # no dependency on the on-chip k-norm chain (rk multiply is a no-op there)
    nk = null_kv[0]
    nk = nk / np.maximum(np.linalg.norm(nk, axis=1, keepdims=True), 1e-12)
    nullkT = np.ascontiguousarray(nk.T.astype(bf16))
    nullv = np.ascontiguousarray(null_kv[1].astype(bf16))
    xkT_bf = [np.ascontiguousarray(x[bi].T.astype(bf16)) for bi in range(b)]

    in_maps = []
    row_sets = []
    for c in range(8):
        bi, qc = c // 4, c % 4
        blocks = [qc, 4 + qc, 8 + qc, 12 + qc]
        rows = np.concatenate([np.arange(P * t, P * t + P) for t in blocks])
        row_sets.append((bi, rows))
        thresh = np.where(rows < 64, 66, rows + 3).astype(np.float32) + 126.0
        in_maps.append({
            "xkT": xkT_bf[bi],
            "xqTb": np.ascontiguousarray(x[bi][rows].T.astype(bf16)),
            "wq": wq_eff,
            "wkv": wkv_bf,
            "wout": wout_bf,
            "thresh": thresh,
            "comb128": comb128,
            "nullkT": nullkT,
            "nullv": nullv,
            "iota": iota,
        })

    nc = _get_nc()
    # Tracing (for the HW-exec-time print) only when explicitly requested:
    # it adds substantial host-side post-processing to each run.
    import os
    trace = bool(os.environ.get("BASS_KERNEL_TRACE")) and _install_ntff_hook()
    try:
        res = run_bass_kernel_spmd(nc, in_maps, core_ids=list(range(8)), trace=trace)
    except (ImportError, ModuleNotFoundError):
        res = run_bass_kernel_spmd(nc, in_maps, core_ids=list(range(8)), trace=False)
    if getattr(res, "exec_time_ns", None) is not None:
        print(f"HW exec time: {res.exec_time_ns} ns")
    out = np.empty((b, n, d), dtype=np.float32)
    for c in range(8):
        bi, rows = row_sets[c]
        out[bi][rows] = res.results[c]["out"]
    return out
